# revision 6
# baseline (speedup 1.0000x reference)
"""Multi-head attention (4x2048x1024, 16 heads) on 8 TRN2 NeuronCores.

v3 sharding: core c = (batch b=c//2, head-group g=c%2 of 8 heads).
Each core projects q/k/v for its 8 heads over the full 2048-token
sequence of its batch (no redundant K/V compute), runs attention for
its heads (4 head-pairs x 2 query-halves, each a 16-key-tile softmax
pipeline identical to v2's), then exchanges attention outputs with its
peer (other head-group, same batch) via per-head-pair 2-rank AllGather
DRAM collectives overlapped with the remaining attention work. A
data-driven 0/1 mask input keeps the program SPMD-symmetric: after the
rank-ordered AllGather each core selects its own token half with two
DVE passes instead of a rank-dependent slice. Output projection
contracts all 16 heads' dims for the core's 1024 tokens, with the
last-exchanged tiles accumulated last so the final collective hides
behind the first six tiles' matmuls.
"""

import numpy as np

import concourse.mybir as mybir
import concourse.tile as tile
from concourse import bacc
from concourse.bass_utils import run_bass_kernel_spmd
FP32 = mybir.dt.float32
BF16 = mybir.dt.bfloat16

DIM = 1024
HEADS = 16
HPC = 8            # heads per core
HD = 64
AUG = HD + 1       # V columns per head + ones column for sum-exp
SCALE = DIM ** -0.5
SEQ = 2048
NQ = 2048          # queries per core (full batch sequence)
NH = 1024          # queries per attention unit (half)
NJ = 2048          # keys per core
B = 4
N_CORES = 8
P = 128
ND = DIM // P      # 8 contraction tiles
NE = HPC * HD // P  # 4 e-tiles (2 heads each)
NBUF = 8           # unit-0 jts with SBUF-buffered exps
GROUPS = [[0, 1], [2, 3], [4, 5], [6, 7]]

TRACE = False
LAST_RESULTS = None
_NC_CACHE = None


def _build():
    nc = bacc.Bacc(
        "TRN2",
        target_bir_lowering=False,
        debug=False,
        enable_asserts=False,
        num_devices=N_CORES,
    )
    # all inputs pre-cast/transposed by host
    xT = nc.dram_tensor("xT", [DIM, NQ], BF16, kind="ExternalInput")
    # my group's w slices: [1024, 3*512] = [q 512 | k 512 | v 512]
    wqkvT = nc.dram_tensor("wqkvT", [DIM, 3 * 512], BF16, kind="ExternalInput")
    woutT = nc.dram_tensor("woutT", [DIM, DIM], BF16, kind="ExternalInput")
    bout = nc.dram_tensor("bout", [1, DIM], FP32, kind="ExternalInput")
    mask = nc.dram_tensor("mask", [1, 2], FP32, kind="ExternalInput")
    out = nc.dram_tensor("out", [NH, DIM], FP32, kind="ExternalOutput")
    # collective bounce buffers, one pair per head-pair
    ccin = [nc.dram_tensor(f"ccin{hp}", [P, NQ], BF16) for hp in range(NE)]
    ccout = [nc.dram_tensor(f"ccout{hp}", [2 * P, NQ], BF16) for hp in range(NE)]

    with tile.TileContext(nc) as tc:
        with (
            tc.tile_pool(name="persist", bufs=1) as persist,
            tc.tile_pool(name="sb", bufs=3) as sb,
            tc.tile_pool(name="small", bufs=3) as small,
        ):
            e0pool_cm = tc.tile_pool(name="e0pool", bufs=1)
            e0pool = e0pool_cm.__enter__()
            xpool_cm = tc.tile_pool(name="xpool", bufs=1)
            xpool = xpool_cm.__enter__()
            wpool_cm = tc.tile_pool(name="wpool", bufs=1)
            wpool = wpool_cm.__enter__()
            pp_cm = tc.tile_pool(name="pp", bufs=3, space="PSUM")
            pp = pp_cm.__enter__()

            # ---- bias broadcast [1,1024] -> [128,1024]; mask scalars
            bias_sb = small.tile([1, DIM], FP32, tag="bias", name="bias", bufs=1)
            nc.sync.dma_start(out=bias_sb, in_=bout.ap())
            bias_bc = small.tile([P, DIM], FP32, tag="biasbc", name="biasbc", bufs=1)
            nc.gpsimd.partition_broadcast(bias_bc, bias_sb)
            mrow = small.tile([1, 2], FP32, tag="mrow", name="mrow", bufs=1)
            nc.sync.dma_start(out=mrow, in_=mask.ap())
            mbc = small.tile([P, 2], FP32, tag="mbc", name="mbc", bufs=1)
            nc.gpsimd.partition_broadcast(mbc, mrow)

            # ---- persistent tiles
            xbf = [xpool.tile([P, NQ], BF16, tag=f"xbf{dt}", name=f"xbf{dt}")
                   for dt in range(ND)]
            qt = [persist.tile([P, NQ], BF16, tag=f"qt{e}", name=f"qt{e}")
                  for e in range(NE)]
            kt = [persist.tile([P, NJ], BF16, tag=f"kt{e}", name=f"kt{e}")
                  for e in range(NE)]
            vaug = [persist.tile([P, HPC * AUG], BF16, tag=f"va{j}",
                                 name=f"va{j}") for j in range(16)]

            wv = [wpool.tile([P, 512], BF16, tag=f"wv{dt}", name=f"wv{dt}")
                  for dt in range(ND)]

            def load_we(ebase, et):
                """[128 p, 8 dt, 128 cols]: one strided DMA per e-tile."""
                w = wpool.tile([P, ND, P], BF16, tag="we", name="we", bufs=4)
                nc.sync.dma_start(
                    out=w,
                    in_=wqkvT.ap()
                    .rearrange("(dt p) e -> p dt e", p=P)
                    [:, :, ebase + et * P: ebase + (et + 1) * P],
                )
                return w

            wq0 = load_we(0, 0)
            wk0 = load_we(512, 0)
            for dt in range(ND):
                nc.sync.dma_start(
                    out=xbf[dt], in_=xT.ap()[dt * P:(dt + 1) * P, :])
            for dt in range(ND):
                nc.sync.dma_start(
                    out=wv[dt],
                    in_=wqkvT.ap()[dt * P:(dt + 1) * P, 2 * 512:3 * 512])

            # ones columns of vaug (sum-exp trick)
            for jt in range(16):
                v3 = vaug[jt].rearrange("p (h c) -> p h c", c=AUG)
                nc.vector.memset(v3[:, :, HD:AUG], 1.0)

            def qk_proj(we, tiles, et, chunks):
                """Project one e-tile (2 heads) for the given seq chunks."""
                pss = {ch: pp.tile([P, DIM], FP32, tag="pp", name="pp")
                       for ch in chunks}
                for dt in range(ND):
                    for ch in chunks:
                        for sc in range(2):
                            nb = ch * DIM + sc * 512
                            nc.tensor.matmul(
                                pss[ch][:, sc * 512:(sc + 1) * 512],
                                we[:, dt, :],
                                xbf[dt][:, nb:nb + 512],
                                start=(dt == 0),
                                stop=(dt == ND - 1),
                            )
                for ch in chunks:
                    dst = tiles[et][:, ch * DIM:(ch + 1) * DIM]
                    if (et + ch) % 2 == 0:
                        nc.vector.tensor_copy(dst, pss[ch])
                    else:
                        nc.scalar.copy(dst, pss[ch])

            def v_proj(jt):
                ps = pp.tile([P, 512], FP32, tag="ppv", name="ppv", bufs=2)
                for dt in range(ND):
                    nc.tensor.matmul(
                        ps,
                        xbf[dt][:, jt * P:(jt + 1) * P],
                        wv[dt],
                        start=(dt == 0),
                        stop=(dt == ND - 1),
                    )
                vsrc = ps.rearrange("p (h c) -> p h c", c=HD)
                vdst = vaug[jt].rearrange("p (h c) -> p h c", c=AUG)[:, :, 0:HD]
                if jt % 2 == 0:
                    nc.vector.tensor_copy(vdst, vsrc)
                else:
                    nc.scalar.copy(vdst, vsrc)

            def emit_dots(pool, tag, hp, jt, half, qh):
                """One head's dots [128 keys, 1024 queries] for key-tile jt,
                query-half qh. half 0 -> PE rows 0:64, half 1 -> rows 64:128."""
                d = pool.tile([P, NH], FP32, tag=tag, name="dots")
                jsl = slice(jt * P, (jt + 1) * P)
                rsl = slice(0, HD) if half == 0 else slice(HD, P)
                for ic in range(2):
                    isl = slice(ic * 512, (ic + 1) * 512)
                    qsl = slice(qh * NH + ic * 512, qh * NH + (ic + 1) * 512)
                    nc.tensor.matmul(
                        d[:, isl], kt[hp][rsl, jsl], qt[hp][rsl, qsl],
                        start=True, stop=True,
                    )
                return d

            def emit_exp(d, e_t):
                nc.scalar.activation(e_t, d, mybir.ActivationFunctionType.Exp,
                                     scale=SCALE)

            # ---- projection phase, with unit-0 (hp0, qh0) dots+exp
            # interleaved; exps buffered in SBUF for AV replay later.
            qk_proj(wq0, qt, 0, [0])
            qk_proj(wk0, kt, 0, [0])
            qk_proj(wk0, kt, 0, [1])

            e0 = [e0pool.tile([P, NH], BF16, tag=f"e0_{j}", name=f"e0_{j}")
                  for j in range(2 * NBUF)]
            # remaining weight-load jobs: q et1-3, k et1-3
            wjobs = [(0, e) for e in range(1, NE)] + \
                    [(512, e) for e in range(1, NE)]
            # proj chunk-jobs consumed by the filler scheduler below:
            # q et0 ch1 first (unit 1 = (hp0, qh1) needs it early).
            pjobs = [(0, 0, 1)]
            for (ebase, e) in wjobs:
                pjobs.append((ebase, e, 0))
                pjobs.append((ebase, e, 1))

            wtiles = {(0, 0): wq0, (512, 0): wk0}
            for key in wjobs:
                wtiles[key] = None
            wtiles[wjobs[0]] = load_we(*wjobs[0])

            # unit-0: dots+exp for jt 0..15 interleaved with v_proj and
            # q/k e-tile projections (PE fills while ACT chews exps).
            widx = 1
            for jt in range(16):
                if jt < NBUF:
                    dA = emit_dots(pp, 'pp', 0, jt, 0, 0)
                    dB = emit_dots(pp, 'pp', 0, jt, 1, 0)
                    emit_exp(dA, e0[2 * jt])
                    emit_exp(dB, e0[2 * jt + 1])
                v_proj(jt)
                if jt < len(pjobs):
                    ebase, e, ch = pjobs[jt]
                    w = wtiles[(ebase, e)]
                    if w is None:
                        w = load_we(ebase, e)
                        wtiles[(ebase, e)] = w
                    if widx < len(wjobs) and wtiles[wjobs[widx]] is None:
                        wtiles[wjobs[widx]] = load_we(*wjobs[widx])
                        widx += 1
                    tiles = qt if ebase == 0 else kt
                    qk_proj(w, tiles, e, [ch])

            pp_cm.__exit__(None, None, None)
            wpool_cm.__exit__(None, None, None)
            xpool_cm.__exit__(None, None, None)

            # ---- attention-phase pools
            late_cm = tc.tile_pool(name="late", bufs=1)
            late = late_cm.__enter__()
            aot = [late.tile([P, NQ], BF16, tag=f"ao{e}", name=f"ao{e}")
                   for e in range(NE)]
            aoX = [late.tile([P, NH], BF16, tag=f"ax{e}", name=f"ax{e}")
                   for e in range(2 * NE)]
            wo = [late.tile([P, DIM], BF16, tag=f"wo{dt}", name=f"wo{dt}")
                  for dt in range(ND)]
            for dt in range(ND):
                nc.sync.dma_start(
                    out=wo[dt], in_=woutT.ap()[dt * P:(dt + 1) * P, :])

            norm_cm = tc.tile_pool(name="norm", bufs=2)
            norm = norm_cm.__enter__()
            pd_cm = tc.tile_pool(name="pd", bufs=2, space="PSUM")
            pd = pd_cm.__enter__()
            pav_cm = tc.tile_pool(name="pav", bufs=2, space="PSUM")
            pav = pav_cm.__enter__()

            def av_accum(av, e_t, jt, head):
                first, last = jt == 0, jt == 15
                for ic in range(2):
                    isl = slice(ic * 512, (ic + 1) * 512)
                    nc.tensor.matmul(
                        av[:, isl],
                        vaug[jt][:, head * AUG:(head + 1) * AUG],
                        e_t[:, isl],
                        start=first, stop=last,
                    )

            def heat(hp, n=10):
                # junk matmuls bridge the unit-boundary PE gap (HW DVFS).
                pb = pd.tile([P, NH], FP32, tag="pd", name="heat")
                for i in range(n):
                    nc.tensor.matmul(
                        pb[:, (i % 2) * 512:(i % 2) * 512 + 512],
                        kt[hp][:, 0:P], qt[hp][:, 0:512],
                        start=True, stop=True,
                    )

            def normalize(hp, qh, avA, avB):
                # per-unit softmax normalization (overlaps next unit).
                osl = slice(qh * NH, (qh + 1) * NH)
                g = norm.tile([33, NH], FP32, tag="g", name="g")
                nc.vector.tensor_copy(g[0:1, :], avA[HD:AUG, :])
                nc.vector.tensor_copy(g[32:33, :], avB[HD:AUG, :])
                nc.vector.tensor_copy(aot[hp][0:HD, osl], avA[0:HD, :])
                nc.vector.tensor_copy(aot[hp][HD:P, osl], avB[0:HD, :])
                rp = norm.tile([33, NH], FP32, tag="rp", name="rp")
                nc.vector.reciprocal_approx_fast(rp, g)
                rbA = norm.tile([P, NH], FP32, tag="rb", name="rb")
                nc.gpsimd.partition_broadcast(rbA, rp[0:1, :])
                nc.vector.tensor_mul(
                    aot[hp][0:HD, osl], aot[hp][0:HD, osl], rbA[0:HD, :])
                tb = norm.tile([1, NH], FP32, tag="tb", name="tb")
                nc.vector.tensor_copy(tb, rp[32:33, :])
                rbB = norm.tile([P, NH], FP32, tag="rb", name="rb")
                nc.gpsimd.partition_broadcast(rbB, tb)
                nc.vector.tensor_mul(
                    aot[hp][HD:P, osl], aot[hp][HD:P, osl], rbB[HD:P, :])

            def exchange(hp):
                # peer exchange of this head-pair's attention outputs:
                # AllGather (rank-ordered) then data-driven half select.
                nc.sync.dma_start(out=ccin[hp].ap(), in_=aot[hp])
                nc.gpsimd.collective_compute(
                    "AllGather", mybir.AluOpType.bypass,
                    replica_groups=GROUPS,
                    ins=[ccin[hp].ap()], outs=[ccout[hp].ap()],
                )
                for gg in range(2):
                    gx = sb.tile([P, NQ], BF16, tag="gx", name="gx", bufs=2)
                    nc.sync.dma_start(
                        out=gx, in_=ccout[hp].ap()[gg * P:(gg + 1) * P, :])
                    e = gg * NE + hp
                    t = sb.tile([P, NH], FP32, tag="selt", name="selt", bufs=2)
                    nc.vector.tensor_scalar(
                        t, gx[:, 0:NH], mbc[:, 0:1], None,
                        mybir.AluOpType.mult)
                    nc.vector.scalar_tensor_tensor(
                        aoX[e], gx[:, NH:NQ], mbc[:, 1:2], t,
                        mybir.AluOpType.mult, mybir.AluOpType.add)

            # ---- attention phase: 8 units = (hp, qh)
            # unit-0 (hp0, qh0): AV replay of buffered jts + streamed jts.
            avA = pav.tile([AUG, NH], FP32, tag="pav", name="av")
            avB = pav.tile([AUG, NH], FP32, tag="pav", name="av")
            dA = emit_dots(pd, 'pd', 0, NBUF, 0, 0)
            dB = emit_dots(pd, 'pd', 0, NBUF, 1, 0)
            av_accum(avA, e0[0], 0, 0)
            av_accum(avB, e0[1], 0, 1)
            for jt in range(NBUF, 16):
                eA = sb.tile([P, NH], BF16, tag="expT", name="expT", bufs=4)
                eB = sb.tile([P, NH], BF16, tag="expT", name="expT", bufs=4)
                emit_exp(dA, eA)
                emit_exp(dB, eB)
                r = jt - NBUF + 1  # replay index
                if jt < 15:
                    dA = emit_dots(pd, 'pd', 0, jt + 1, 0, 0)
                if r < NBUF:
                    av_accum(avA, e0[2 * r], r, 0)
                    av_accum(avB, e0[2 * r + 1], r, 1)
                av_accum(avA, eA, jt, 0)
                if jt < 15:
                    dB = emit_dots(pd, 'pd', 0, jt + 1, 1, 0)
                av_accum(avB, eB, jt, 1)
            normalize(0, 0, avA, avB)

            # units 1-7: lookahead-dots pipeline
            for u in range(1, 8):
                hp, qh = u // 2, u % 2
                avA = pav.tile([AUG, NH], FP32, tag="pav", name="av")
                avB = pav.tile([AUG, NH], FP32, tag="pav", name="av")
                dA = emit_dots(pd, 'pd', hp, 0, 0, qh)
                dB = emit_dots(pd, 'pd', hp, 0, 1, qh)
                heat(hp)
                for jt in range(16):
                    eA = sb.tile([P, NH], BF16, tag="expT", name="expT", bufs=4)
                    eB = sb.tile([P, NH], BF16, tag="expT", name="expT", bufs=4)
                    emit_exp(dA, eA)
                    emit_exp(dB, eB)
                    if jt < 15:
                        dA = emit_dots(pd, 'pd', hp, jt + 1, 0, qh)
                    av_accum(avA, eA, jt, 2 * hp)
                    if jt < 15:
                        dB = emit_dots(pd, 'pd', hp, jt + 1, 1, qh)
                    av_accum(avB, eB, jt, 2 * hp + 1)
                normalize(hp, qh, avA, avB)
                if qh == 1:
                    exchange(hp)

            pav_cm.__exit__(None, None, None)
            pd_cm.__exit__(None, None, None)
            norm_cm.__exit__(None, None, None)

            # ---- output projection + bias for my 1024 tokens.
            # aoX order: e = g*4 + hp ; wout rows for ao dim block e are
            # woutT rows [g*512 + hp*128 ...]. Accumulate hp3 tiles (3, 7)
            # last so the final exchange hides behind the others.
            et_order = [0, 1, 2, 4, 5, 6, 3, 7]
            po_cm = tc.tile_pool(name="po", bufs=8, space="PSUM")
            po = po_cm.__enter__()

            def wo_row(e):
                # ao dim-tile e = g*4+hp -> wout row-tile g*4+hp (same index)
                return wo[e]

            for fc in range(2):
                fsl = slice(fc * 512, (fc + 1) * 512)
                pss = {}
                for it in range(8):
                    pss[it] = po.tile([P, 512], FP32, tag="po", name="po")
                    for ei in range(6):
                        e = et_order[ei]
                        nc.tensor.matmul(
                            pss[it],
                            aoX[e][:, it * P:(it + 1) * P],
                            wo_row(e)[:, fsl],
                            start=(ei == 0),
                            stop=False,
                        )
                for it in range(8):
                    for ei in range(6, 8):
                        e = et_order[ei]
                        nc.tensor.matmul(
                            pss[it],
                            aoX[e][:, it * P:(it + 1) * P],
                            wo_row(e)[:, fsl],
                            start=False,
                            stop=(ei == 7),
                        )
                    osb = sb.tile([P, 512], FP32, tag="outsb", name="outsb",
                                  bufs=4)
                    nc.vector.tensor_add(osb, pss[it], bias_bc[:, fsl])
                    nc.sync.dma_start(
                        out=out.ap()[it * P:(it + 1) * P, fsl], in_=osb)
            po_cm.__exit__(None, None, None)
            late_cm.__exit__(None, None, None)
            e0pool_cm.__exit__(None, None, None)

    nc.compile()
    return nc


def _get_nc():
    global _NC_CACHE
    if _NC_CACHE is None:
        _NC_CACHE = _build()
    return _NC_CACHE


def kernel(x, w_qkv, w_out, b_out):
    global LAST_RESULTS
    import ml_dtypes
    BF = ml_dtypes.bfloat16
    x = np.asarray(x, dtype=np.float32)
    w_qkv = np.asarray(w_qkv, dtype=np.float32)
    w_out = np.asarray(w_out, dtype=np.float32)
    b_out = np.asarray(b_out, dtype=np.float32)

    nc = _get_nc()

    wqkvT_full = w_qkv.T.astype(BF)  # [1024, 3072]
    woutT = np.ascontiguousarray(w_out.T.astype(BF))
    brow = np.ascontiguousarray(b_out.reshape(1, DIM))

    in_maps = []
    for c in range(N_CORES):
        b, g = divmod(c, 2)
        gsl = slice(g * 512, (g + 1) * 512)
        wslice = np.ascontiguousarray(np.concatenate([
            wqkvT_full[:, 0 * DIM:1 * DIM][:, gsl],
            wqkvT_full[:, 1 * DIM:2 * DIM][:, gsl],
            wqkvT_full[:, 2 * DIM:3 * DIM][:, gsl],
        ], axis=1))
        xTc = np.ascontiguousarray(x[b].T.astype(BF))
        m = np.zeros((1, 2), np.float32)
        m[0, g] = 1.0
        in_maps.append({
            "xT": xTc,
            "wqkvT": wslice,
            "woutT": woutT,
            "bout": brow,
            "mask": m,
        })

    res = run_bass_kernel_spmd(
        nc, in_maps, core_ids=list(range(N_CORES)), trace=TRACE
    )
    LAST_RESULTS = res

    out = np.empty((B, SEQ, DIM), dtype=np.float32)
    for c in range(N_CORES):
        b, g = divmod(c, 2)
        out[b, g * NH:(g + 1) * NH, :] = res.results[c]["out"]
    return out


# revision 8
# speedup vs baseline: 1.0676x; 1.0676x over previous
"""Multi-head attention (4x2048x1024, 16 heads) on 8 TRN2 NeuronCores.

v3 sharding: core c = (batch b=c//2, head-group g=c%2 of 8 heads).
Each core projects q/k/v for its 8 heads over the full 2048-token
sequence of its batch (no redundant K/V compute), runs attention for
its heads (4 head-pairs x 2 query-halves, each a 16-key-tile softmax
pipeline identical to v2's), then exchanges attention outputs with its
peer (other head-group, same batch) via per-head-pair 2-rank AllGather
DRAM collectives overlapped with the remaining attention work. A
data-driven 0/1 mask input keeps the program SPMD-symmetric: after the
rank-ordered AllGather each core selects its own token half with two
DVE passes instead of a rank-dependent slice. Output projection
contracts all 16 heads' dims for the core's 1024 tokens, with the
last-exchanged tiles accumulated last so the final collective hides
behind the first six tiles' matmuls.
"""

import numpy as np

import concourse.mybir as mybir
import concourse.tile as tile
from concourse import bacc
from concourse.bass_utils import run_bass_kernel_spmd
FP32 = mybir.dt.float32
BF16 = mybir.dt.bfloat16

DIM = 1024
HEADS = 16
HPC = 8            # heads per core
HD = 64
AUG = HD + 1       # V columns per head + ones column for sum-exp
SCALE = DIM ** -0.5
SEQ = 2048
NQ = 2048          # queries per core (full batch sequence)
NH = 1024          # queries per attention unit (half)
NJ = 2048          # keys per core
B = 4
N_CORES = 8
P = 128
ND = DIM // P      # 8 contraction tiles
NE = HPC * HD // P  # 4 e-tiles (2 heads each)
NBUF = 8           # unit-0 jts with SBUF-buffered exps
GROUPS = [[0, 1], [2, 3], [4, 5], [6, 7]]

TRACE = False
LAST_RESULTS = None
_NC_CACHE = None


def _build():
    nc = bacc.Bacc(
        "TRN2",
        target_bir_lowering=False,
        debug=False,
        enable_asserts=False,
        num_devices=N_CORES,
    )
    # all inputs pre-cast/transposed by host
    xT = nc.dram_tensor("xT", [DIM, NQ], BF16, kind="ExternalInput")
    # my group's w slices: [1024, 3*512] = [q 512 | k 512 | v 512]
    wqkvT = nc.dram_tensor("wqkvT", [DIM, 3 * 512], BF16, kind="ExternalInput")
    woutT = nc.dram_tensor("woutT", [DIM, DIM], BF16, kind="ExternalInput")
    bout = nc.dram_tensor("bout", [1, DIM], FP32, kind="ExternalInput")
    mask = nc.dram_tensor("mask", [1, 2], FP32, kind="ExternalInput")
    out = nc.dram_tensor("out", [NH, DIM], FP32, kind="ExternalOutput")
    # collective bounce buffers, one pair per head-pair
    ccin = [nc.dram_tensor(f"ccin{hp}", [P, NQ], BF16) for hp in range(NE)]
    ccout = [nc.dram_tensor(f"ccout{hp}", [2 * P, NQ], BF16) for hp in range(NE)]

    with tile.TileContext(nc) as tc:
        with (
            tc.tile_pool(name="persist", bufs=1) as persist,
            tc.tile_pool(name="sb", bufs=3) as sb,
            tc.tile_pool(name="small", bufs=3) as small,
        ):
            e0pool_cm = tc.tile_pool(name="e0pool", bufs=1)
            e0pool = e0pool_cm.__enter__()
            xpool_cm = tc.tile_pool(name="xpool", bufs=1)
            xpool = xpool_cm.__enter__()
            wpool_cm = tc.tile_pool(name="wpool", bufs=1)
            wpool = wpool_cm.__enter__()
            pp_cm = tc.tile_pool(name="pp", bufs=3, space="PSUM")
            pp = pp_cm.__enter__()

            # ---- bias broadcast [1,1024] -> [128,1024]; mask scalars
            bias_sb = small.tile([1, DIM], FP32, tag="bias", name="bias", bufs=1)
            nc.sync.dma_start(out=bias_sb, in_=bout.ap())
            bias_bc = small.tile([P, DIM], FP32, tag="biasbc", name="biasbc", bufs=1)
            nc.gpsimd.partition_broadcast(bias_bc, bias_sb)
            mrow = small.tile([1, 2], FP32, tag="mrow", name="mrow", bufs=1)
            nc.sync.dma_start(out=mrow, in_=mask.ap())
            mbc = small.tile([P, 2], FP32, tag="mbc", name="mbc", bufs=1)
            nc.gpsimd.partition_broadcast(mbc, mrow)

            # ---- persistent tiles
            xbf = [xpool.tile([P, NQ], BF16, tag=f"xbf{dt}", name=f"xbf{dt}")
                   for dt in range(ND)]
            qt = [persist.tile([P, NQ], BF16, tag=f"qt{e}", name=f"qt{e}")
                  for e in range(NE)]
            kt = [persist.tile([P, NJ], BF16, tag=f"kt{e}", name=f"kt{e}")
                  for e in range(NE)]
            vaug = [persist.tile([P, HPC * AUG], BF16, tag=f"va{j}",
                                 name=f"va{j}") for j in range(16)]

            wv = [wpool.tile([P, 512], BF16, tag=f"wv{dt}", name=f"wv{dt}")
                  for dt in range(ND)]

            def load_we(ebase, et):
                """[128 p, 8 dt, 128 cols]: one strided DMA per e-tile."""
                w = wpool.tile([P, ND, P], BF16, tag="we", name="we", bufs=4)
                nc.sync.dma_start(
                    out=w,
                    in_=wqkvT.ap()
                    .rearrange("(dt p) e -> p dt e", p=P)
                    [:, :, ebase + et * P: ebase + (et + 1) * P],
                )
                return w

            wq0 = load_we(0, 0)
            wk0 = load_we(512, 0)
            for dt in range(ND):
                nc.sync.dma_start(
                    out=xbf[dt], in_=xT.ap()[dt * P:(dt + 1) * P, :])
            for dt in range(ND):
                nc.sync.dma_start(
                    out=wv[dt],
                    in_=wqkvT.ap()[dt * P:(dt + 1) * P, 2 * 512:3 * 512])

            # ones columns of vaug (sum-exp trick)
            for jt in range(16):
                v3 = vaug[jt].rearrange("p (h c) -> p h c", c=AUG)
                nc.vector.memset(v3[:, :, HD:AUG], 1.0)

            def qk_proj(we, tiles, et, chunks):
                """Project one e-tile (2 heads) for the given seq chunks."""
                pss = {ch: pp.tile([P, DIM], FP32, tag="pp", name="pp")
                       for ch in chunks}
                for dt in range(ND):
                    for ch in chunks:
                        for sc in range(2):
                            nb = ch * DIM + sc * 512
                            nc.tensor.matmul(
                                pss[ch][:, sc * 512:(sc + 1) * 512],
                                we[:, dt, :],
                                xbf[dt][:, nb:nb + 512],
                                start=(dt == 0),
                                stop=(dt == ND - 1),
                            )
                for ch in chunks:
                    dst = tiles[et][:, ch * DIM:(ch + 1) * DIM]
                    if (et + ch) % 2 == 0:
                        nc.vector.tensor_copy(dst, pss[ch])
                    else:
                        nc.scalar.copy(dst, pss[ch])

            def v_proj(jt):
                ps = pp.tile([P, 512], FP32, tag="ppv", name="ppv", bufs=2)
                for dt in range(ND):
                    nc.tensor.matmul(
                        ps,
                        xbf[dt][:, jt * P:(jt + 1) * P],
                        wv[dt],
                        start=(dt == 0),
                        stop=(dt == ND - 1),
                    )
                vsrc = ps.rearrange("p (h c) -> p h c", c=HD)
                vdst = vaug[jt].rearrange("p (h c) -> p h c", c=AUG)[:, :, 0:HD]
                if jt % 2 == 0:
                    nc.vector.tensor_copy(vdst, vsrc)
                else:
                    nc.scalar.copy(vdst, vsrc)

            def emit_dots(pool, tag, hp, jt, half, qh):
                """One head's dots [128 keys, 1024 queries] for key-tile jt,
                query-half qh. half 0 -> PE rows 0:64, half 1 -> rows 64:128."""
                d = pool.tile([P, NH], FP32, tag=tag, name="dots")
                jsl = slice(jt * P, (jt + 1) * P)
                rsl = slice(0, HD) if half == 0 else slice(HD, P)
                for ic in range(2):
                    isl = slice(ic * 512, (ic + 1) * 512)
                    qsl = slice(qh * NH + ic * 512, qh * NH + (ic + 1) * 512)
                    nc.tensor.matmul(
                        d[:, isl], kt[hp][rsl, jsl], qt[hp][rsl, qsl],
                        start=True, stop=True,
                    )
                return d

            def emit_exp(d, e_t):
                nc.scalar.activation(e_t, d, mybir.ActivationFunctionType.Exp,
                                     scale=SCALE)

            # ---- projection phase, with unit-0 (hp0, qh0) dots+exp
            # interleaved; exps buffered in SBUF for AV replay later.
            qk_proj(wq0, qt, 0, [0])
            qk_proj(wk0, kt, 0, [0])
            qk_proj(wk0, kt, 0, [1])

            e0 = [e0pool.tile([P, NH], BF16, tag=f"e0_{j}", name=f"e0_{j}")
                  for j in range(2 * NBUF)]
            # remaining weight-load jobs: q et1-3, k et1-3
            wjobs = [(0, e) for e in range(1, NE)] + \
                    [(512, e) for e in range(1, NE)]
            # proj chunk-jobs consumed by the filler scheduler below:
            # q et0 ch1 first (unit 1 = (hp0, qh1) needs it early).
            pjobs = [(0, 0, 1)]
            for (ebase, e) in wjobs:
                pjobs.append((ebase, e, 0))
                pjobs.append((ebase, e, 1))

            wtiles = {(0, 0): wq0, (512, 0): wk0}
            for key in wjobs:
                wtiles[key] = None
            wtiles[wjobs[0]] = load_we(*wjobs[0])

            # unit-0: dots+exp for jt 0..15 interleaved with v_proj and
            # q/k e-tile projections (PE fills while ACT chews exps).
            widx = 1
            for jt in range(16):
                if jt < NBUF:
                    dA = emit_dots(pp, 'pp', 0, jt, 0, 0)
                    dB = emit_dots(pp, 'pp', 0, jt, 1, 0)
                    emit_exp(dA, e0[2 * jt])
                    emit_exp(dB, e0[2 * jt + 1])
                v_proj(jt)
                if jt < len(pjobs):
                    ebase, e, ch = pjobs[jt]
                    w = wtiles[(ebase, e)]
                    if w is None:
                        w = load_we(ebase, e)
                        wtiles[(ebase, e)] = w
                    if widx < len(wjobs) and wtiles[wjobs[widx]] is None:
                        wtiles[wjobs[widx]] = load_we(*wjobs[widx])
                        widx += 1
                    tiles = qt if ebase == 0 else kt
                    qk_proj(w, tiles, e, [ch])

            pp_cm.__exit__(None, None, None)
            wpool_cm.__exit__(None, None, None)
            xpool_cm.__exit__(None, None, None)

            # ---- attention-phase pools
            late_cm = tc.tile_pool(name="late", bufs=1)
            late = late_cm.__enter__()
            aot = [late.tile([P, NQ], BF16, tag=f"ao{e}", name=f"ao{e}")
                   for e in range(NE)]
            aoX = [late.tile([P, NH], BF16, tag=f"ax{e}", name=f"ax{e}")
                   for e in range(2 * NE)]
            wo = [late.tile([P, DIM], BF16, tag=f"wo{dt}", name=f"wo{dt}")
                  for dt in range(ND)]
            for dt in range(ND):
                nc.sync.dma_start(
                    out=wo[dt], in_=woutT.ap()[dt * P:(dt + 1) * P, :])

            norm_cm = tc.tile_pool(name="norm", bufs=2)
            norm = norm_cm.__enter__()
            pd_cm = tc.tile_pool(name="pd", bufs=2, space="PSUM")
            pd = pd_cm.__enter__()
            pav_cm = tc.tile_pool(name="pav", bufs=2, space="PSUM")
            pav = pav_cm.__enter__()

            def av_accum(av, e_t, jt, head):
                first, last = jt == 0, jt == 15
                for ic in range(2):
                    isl = slice(ic * 512, (ic + 1) * 512)
                    nc.tensor.matmul(
                        av[:, isl],
                        vaug[jt][:, head * AUG:(head + 1) * AUG],
                        e_t[:, isl],
                        start=first, stop=last,
                    )

            def heat(hp, n=10):
                # junk matmuls bridge the unit-boundary PE gap (HW DVFS).
                pb = pd.tile([P, NH], FP32, tag="pd", name="heat")
                for i in range(n):
                    nc.tensor.matmul(
                        pb[:, (i % 2) * 512:(i % 2) * 512 + 512],
                        kt[hp][:, 0:P], qt[hp][:, 0:512],
                        start=True, stop=True,
                    )

            def normalize(hp, qh, avA, avB):
                # per-unit softmax normalization (overlaps next unit).
                osl = slice(qh * NH, (qh + 1) * NH)
                g = norm.tile([33, NH], FP32, tag="g", name="g")
                nc.vector.tensor_copy(g[0:1, :], avA[HD:AUG, :])
                nc.vector.tensor_copy(g[32:33, :], avB[HD:AUG, :])
                nc.vector.tensor_copy(aot[hp][0:HD, osl], avA[0:HD, :])
                nc.vector.tensor_copy(aot[hp][HD:P, osl], avB[0:HD, :])
                rp = norm.tile([33, NH], FP32, tag="rp", name="rp")
                nc.vector.reciprocal_approx_fast(rp, g)
                rbA = norm.tile([P, NH], FP32, tag="rb", name="rb")
                nc.gpsimd.partition_broadcast(rbA, rp[0:1, :])
                nc.vector.tensor_mul(
                    aot[hp][0:HD, osl], aot[hp][0:HD, osl], rbA[0:HD, :])
                tb = norm.tile([1, NH], FP32, tag="tb", name="tb")
                nc.vector.tensor_copy(tb, rp[32:33, :])
                rbB = norm.tile([P, NH], FP32, tag="rb", name="rb")
                nc.gpsimd.partition_broadcast(rbB, tb)
                nc.vector.tensor_mul(
                    aot[hp][HD:P, osl], aot[hp][HD:P, osl], rbB[HD:P, :])

            def exchange(hp):
                # peer exchange of this head-pair's attention outputs:
                # rank-ordered AllGather through DRAM. Trigger only — the
                # CC takes ~35us, so the gathered-side DMAs and selects are
                # deferred (select_hp) to ~2 units later; anything queued
                # behind them on DVE/Sync would stall the whole pipeline.
                nc.sync.dma_start(out=ccin[hp].ap(), in_=aot[hp])
                nc.gpsimd.collective_compute(
                    "AllGather", mybir.AluOpType.bypass,
                    replica_groups=GROUPS,
                    ins=[ccin[hp].ap()], outs=[ccout[hp].ap()],
                )

            def select_hp(hp):
                # data-driven token-half select from the gathered buffer
                for gg in range(2):
                    gx = sb.tile([P, NQ], BF16, tag="gx", name="gx", bufs=2)
                    nc.sync.dma_start(
                        out=gx, in_=ccout[hp].ap()[gg * P:(gg + 1) * P, :])
                    e = gg * NE + hp
                    t = sb.tile([P, NH], FP32, tag="selt", name="selt", bufs=2)
                    nc.vector.tensor_scalar(
                        t, gx[:, 0:NH], mbc[:, 0:1], None,
                        mybir.AluOpType.mult)
                    nc.vector.scalar_tensor_tensor(
                        aoX[e], gx[:, NH:NQ], mbc[:, 1:2], t,
                        mybir.AluOpType.mult, mybir.AluOpType.add)

            # ---- attention phase: 8 units = (hp, qh)
            # unit-0 (hp0, qh0): AV replay of buffered jts + streamed jts.
            avA = pav.tile([AUG, NH], FP32, tag="pav", name="av")
            avB = pav.tile([AUG, NH], FP32, tag="pav", name="av")
            dA = emit_dots(pd, 'pd', 0, NBUF, 0, 0)
            dB = emit_dots(pd, 'pd', 0, NBUF, 1, 0)
            av_accum(avA, e0[0], 0, 0)
            av_accum(avB, e0[1], 0, 1)
            for jt in range(NBUF, 16):
                eA = sb.tile([P, NH], BF16, tag="expT", name="expT", bufs=4)
                eB = sb.tile([P, NH], BF16, tag="expT", name="expT", bufs=4)
                emit_exp(dA, eA)
                emit_exp(dB, eB)
                r = jt - NBUF + 1  # replay index
                if jt < 15:
                    dA = emit_dots(pd, 'pd', 0, jt + 1, 0, 0)
                if r < NBUF:
                    av_accum(avA, e0[2 * r], r, 0)
                    av_accum(avB, e0[2 * r + 1], r, 1)
                av_accum(avA, eA, jt, 0)
                if jt < 15:
                    dB = emit_dots(pd, 'pd', 0, jt + 1, 1, 0)
                av_accum(avB, eB, jt, 1)
            normalize(0, 0, avA, avB)

            # units 1-7: lookahead-dots pipeline.
            # CC(hp) fires after unit 2hp+1; its selects run inside unit
            # 2hp+4 (or the tail for hp2/hp3) when the CC is long done.
            sel_at = {4: 0, 6: 1, 7: 2}
            for u in range(1, 8):
                hp, qh = u // 2, u % 2
                avA = pav.tile([AUG, NH], FP32, tag="pav", name="av")
                avB = pav.tile([AUG, NH], FP32, tag="pav", name="av")
                dA = emit_dots(pd, 'pd', hp, 0, 0, qh)
                dB = emit_dots(pd, 'pd', hp, 0, 1, qh)
                heat(hp)
                for jt in range(16):
                    eA = sb.tile([P, NH], BF16, tag="expT", name="expT", bufs=4)
                    eB = sb.tile([P, NH], BF16, tag="expT", name="expT", bufs=4)
                    emit_exp(dA, eA)
                    emit_exp(dB, eB)
                    if jt < 15:
                        dA = emit_dots(pd, 'pd', hp, jt + 1, 0, qh)
                    av_accum(avA, eA, jt, 2 * hp)
                    if jt < 15:
                        dB = emit_dots(pd, 'pd', hp, jt + 1, 1, qh)
                    av_accum(avB, eB, jt, 2 * hp + 1)
                    if jt == 8 and u in sel_at:
                        select_hp(sel_at[u])
                normalize(hp, qh, avA, avB)
                if qh == 1:
                    exchange(hp)
            select_hp(3)

            pav_cm.__exit__(None, None, None)
            pd_cm.__exit__(None, None, None)
            norm_cm.__exit__(None, None, None)

            # ---- output projection + bias for my 1024 tokens.
            # aoX order: e = g*4 + hp ; wout rows for ao dim block e are
            # woutT rows [g*512 + hp*128 ...]. Accumulate hp3 tiles (3, 7)
            # last so the final exchange hides behind the others.
            et_order = [0, 1, 2, 4, 5, 6, 3, 7]
            po_cm = tc.tile_pool(name="po", bufs=8, space="PSUM")
            po = po_cm.__enter__()

            def wo_row(e):
                # ao dim-tile e = g*4+hp -> wout row-tile g*4+hp (same index)
                return wo[e]

            for fc in range(2):
                fsl = slice(fc * 512, (fc + 1) * 512)
                pss = {}
                for it in range(8):
                    pss[it] = po.tile([P, 512], FP32, tag="po", name="po")
                    for ei in range(6):
                        e = et_order[ei]
                        nc.tensor.matmul(
                            pss[it],
                            aoX[e][:, it * P:(it + 1) * P],
                            wo_row(e)[:, fsl],
                            start=(ei == 0),
                            stop=False,
                        )
                for it in range(8):
                    for ei in range(6, 8):
                        e = et_order[ei]
                        nc.tensor.matmul(
                            pss[it],
                            aoX[e][:, it * P:(it + 1) * P],
                            wo_row(e)[:, fsl],
                            start=False,
                            stop=(ei == 7),
                        )
                    osb = sb.tile([P, 512], FP32, tag="outsb", name="outsb",
                                  bufs=4)
                    nc.vector.tensor_add(osb, pss[it], bias_bc[:, fsl])
                    nc.sync.dma_start(
                        out=out.ap()[it * P:(it + 1) * P, fsl], in_=osb)
            po_cm.__exit__(None, None, None)
            late_cm.__exit__(None, None, None)
            e0pool_cm.__exit__(None, None, None)

    nc.compile()
    return nc


def _get_nc():
    global _NC_CACHE
    if _NC_CACHE is None:
        _NC_CACHE = _build()
    return _NC_CACHE


def kernel(x, w_qkv, w_out, b_out):
    global LAST_RESULTS
    import ml_dtypes
    BF = ml_dtypes.bfloat16
    x = np.asarray(x, dtype=np.float32)
    w_qkv = np.asarray(w_qkv, dtype=np.float32)
    w_out = np.asarray(w_out, dtype=np.float32)
    b_out = np.asarray(b_out, dtype=np.float32)

    nc = _get_nc()

    wqkvT_full = w_qkv.T.astype(BF)  # [1024, 3072]
    woutT = np.ascontiguousarray(w_out.T.astype(BF))
    brow = np.ascontiguousarray(b_out.reshape(1, DIM))

    in_maps = []
    for c in range(N_CORES):
        b, g = divmod(c, 2)
        gsl = slice(g * 512, (g + 1) * 512)
        wslice = np.ascontiguousarray(np.concatenate([
            wqkvT_full[:, 0 * DIM:1 * DIM][:, gsl],
            wqkvT_full[:, 1 * DIM:2 * DIM][:, gsl],
            wqkvT_full[:, 2 * DIM:3 * DIM][:, gsl],
        ], axis=1))
        xTc = np.ascontiguousarray(x[b].T.astype(BF))
        m = np.zeros((1, 2), np.float32)
        m[0, g] = 1.0
        in_maps.append({
            "xT": xTc,
            "wqkvT": wslice,
            "woutT": woutT,
            "bout": brow,
            "mask": m,
        })

    res = run_bass_kernel_spmd(
        nc, in_maps, core_ids=list(range(N_CORES)), trace=TRACE
    )
    LAST_RESULTS = res

    out = np.empty((B, SEQ, DIM), dtype=np.float32)
    for c in range(N_CORES):
        b, g = divmod(c, 2)
        out[b, g * NH:(g + 1) * NH, :] = res.results[c]["out"]
    return out


# revision 19
# speedup vs baseline: 1.1659x; 1.0921x over previous
"""Multi-head attention (4x2048x1024, 16 heads) on 8 TRN2 NeuronCores.

v3 sharding: core c = (batch b=c//2, head-group g=c%2 of 8 heads).
Each core projects q/k/v for its 8 heads over the full 2048-token
sequence of its batch (no redundant K/V compute), runs attention for
its heads (4 head-pairs x 2 query-halves, each a 16-key-tile softmax
pipeline identical to v2's), then exchanges attention outputs with its
peer (other head-group, same batch) via per-head-pair 2-rank AllGather
DRAM collectives overlapped with the remaining attention work. A
data-driven 0/1 mask input keeps the program SPMD-symmetric: after the
rank-ordered AllGather each core selects its own token half with two
DVE passes instead of a rank-dependent slice. Output projection
contracts all 16 heads' dims for the core's 1024 tokens, with the
last-exchanged tiles accumulated last so the final collective hides
behind the first six tiles' matmuls.
"""

import numpy as np

import concourse.mybir as mybir
import concourse.tile as tile
from concourse import bacc
from concourse.bass_utils import run_bass_kernel_spmd
FP32 = mybir.dt.float32
BF16 = mybir.dt.bfloat16

DIM = 1024
HEADS = 16
HPC = 8            # heads per core
HD = 64
AUG = HD + 1       # V columns per head + ones column for sum-exp
SCALE = DIM ** -0.5
SEQ = 2048
NQ = 2048          # queries per core (full batch sequence)
NH = 1024          # queries per attention unit (half)
NJ = 2048          # keys per core
B = 4
N_CORES = 8
P = 128
ND = DIM // P      # 8 contraction tiles
NE = HPC * HD // P  # 4 e-tiles (2 heads each)
NBUF = 16          # unit-0 jts with SBUF-buffered exps (all)
GROUPS = [[0, 1], [2, 3], [4, 5], [6, 7]]

TRACE = False
LAST_RESULTS = None
_NC_CACHE = None


def _build():
    nc = bacc.Bacc(
        "TRN2",
        target_bir_lowering=False,
        debug=False,
        enable_asserts=False,
        num_devices=N_CORES,
    )
    # all inputs pre-cast/transposed by host
    xT = nc.dram_tensor("xT", [DIM, NQ], BF16, kind="ExternalInput")
    # my group's w slices: [1024, 3*512] = [q 512 | k 512 | v 512]
    wqkvT = nc.dram_tensor("wqkvT", [DIM, 3 * 512], BF16, kind="ExternalInput")
    woutT = nc.dram_tensor("woutT", [DIM, DIM], BF16, kind="ExternalInput")
    bout = nc.dram_tensor("bout", [1, DIM], FP32, kind="ExternalInput")
    mask = nc.dram_tensor("mask", [1, 2], FP32, kind="ExternalInput")
    out = nc.dram_tensor("out", [NH, DIM], FP32, kind="ExternalOutput")
    # collective bounce buffers, one per (head-pair, query-half)
    ccin = [[nc.dram_tensor(f"ccin{hp}_{qh}", [P, NH], BF16) for qh in range(2)]
            for hp in range(NE)]
    ccout = [[nc.dram_tensor(f"ccout{hp}_{qh}", [2 * P, NH], BF16)
              for qh in range(2)] for hp in range(NE)]
    # tiny warmup collective: the first CC pays ~50us of comm init +
    # cross-core rendezvous; absorb it during the projection phase
    wuin = nc.dram_tensor("wuin", [1, 2], FP32)
    wuout = nc.dram_tensor("wuout", [2, 2], FP32)

    with tile.TileContext(nc) as tc:
        with (
            tc.tile_pool(name="persist", bufs=1) as persist,
            tc.tile_pool(name="sb", bufs=3) as sb,
            tc.tile_pool(name="small", bufs=3) as small,
        ):
            e0pool_cm = tc.tile_pool(name="e0pool", bufs=1)
            e0pool = e0pool_cm.__enter__()
            xpool_cm = tc.tile_pool(name="xpool", bufs=1)
            xpool = xpool_cm.__enter__()
            wpool_cm = tc.tile_pool(name="wpool", bufs=1)
            wpool = wpool_cm.__enter__()
            pp_cm = tc.tile_pool(name="pp", bufs=3, space="PSUM")
            pp = pp_cm.__enter__()

            # ---- bias broadcast [1,1024] -> [128,1024]; mask scalars
            bias_sb = small.tile([1, DIM], FP32, tag="bias", name="bias", bufs=1)
            nc.sync.dma_start(out=bias_sb, in_=bout.ap())
            mrow = small.tile([1, 2], FP32, tag="mrow", name="mrow", bufs=1)
            nc.sync.dma_start(out=mrow, in_=mask.ap())
            mbc = small.tile([P, 2], FP32, tag="mbc", name="mbc", bufs=1)
            nc.gpsimd.partition_broadcast(mbc, mrow)
            # warmup CC, anchored through the bias path (x0 contribution,
            # before the bias broadcast) so it cannot be dead-code-eliminated
            nc.sync.dma_start(out=wuin.ap(), in_=mrow)
            nc.gpsimd.collective_compute(
                "AllGather", mybir.AluOpType.bypass,
                replica_groups=GROUPS,
                ins=[wuin.ap()], outs=[wuout.ap()],
            )
            wu_sb = small.tile([1, 2], FP32, tag="wu", name="wu", bufs=1)
            nc.sync.dma_start(out=wu_sb, in_=wuout.ap()[0:1, :])
            nc.vector.scalar_tensor_tensor(
                bias_sb[0:1, 0:2], wu_sb, 0.0, bias_sb[0:1, 0:2],
                mybir.AluOpType.mult, mybir.AluOpType.add)
            bias_bc = small.tile([P, DIM], FP32, tag="biasbc", name="biasbc", bufs=1)
            nc.gpsimd.partition_broadcast(bias_bc, bias_sb)

            # ---- persistent tiles
            xbf = [xpool.tile([P, NQ], BF16, tag=f"xbf{dt}", name=f"xbf{dt}")
                   for dt in range(ND)]
            qt = [persist.tile([P, NQ], BF16, tag=f"qt{e}", name=f"qt{e}")
                  for e in range(NE)]
            kt = [persist.tile([P, NJ], BF16, tag=f"kt{e}", name=f"kt{e}")
                  for e in range(NE)]
            vaug = [persist.tile([P, HPC * AUG], BF16, tag=f"va{j}",
                                 name=f"va{j}") for j in range(16)]

            wv = [wpool.tile([P, 512], BF16, tag=f"wv{dt}", name=f"wv{dt}")
                  for dt in range(ND)]

            def load_we(ebase, et):
                """[128 p, 8 dt, 128 cols]: one strided DMA per e-tile."""
                w = wpool.tile([P, ND, P], BF16, tag="we", name="we", bufs=4)
                nc.sync.dma_start(
                    out=w,
                    in_=wqkvT.ap()
                    .rearrange("(dt p) e -> p dt e", p=P)
                    [:, :, ebase + et * P: ebase + (et + 1) * P],
                )
                return w

            wq0 = load_we(0, 0)
            wk0 = load_we(512, 0)
            for dt in range(ND):
                nc.sync.dma_start(
                    out=xbf[dt], in_=xT.ap()[dt * P:(dt + 1) * P, :])
            for dt in range(ND):
                nc.sync.dma_start(
                    out=wv[dt],
                    in_=wqkvT.ap()[dt * P:(dt + 1) * P, 2 * 512:3 * 512])

            # ones columns of vaug (sum-exp trick)
            for jt in range(16):
                v3 = vaug[jt].rearrange("p (h c) -> p h c", c=AUG)
                nc.vector.memset(v3[:, :, HD:AUG], 1.0)

            def qk_proj(we, tiles, et, chunks):
                """Project one e-tile (2 heads) for the given seq chunks."""
                pss = {ch: pp.tile([P, DIM], FP32, tag="pp", name="pp")
                       for ch in chunks}
                for dt in range(ND):
                    for ch in chunks:
                        for sc in range(2):
                            nb = ch * DIM + sc * 512
                            nc.tensor.matmul(
                                pss[ch][:, sc * 512:(sc + 1) * 512],
                                we[:, dt, :],
                                xbf[dt][:, nb:nb + 512],
                                start=(dt == 0),
                                stop=(dt == ND - 1),
                            )
                for ch in chunks:
                    dst = tiles[et][:, ch * DIM:(ch + 1) * DIM]
                    if (et + ch) % 2 == 0:
                        nc.vector.tensor_copy(dst, pss[ch])
                    else:
                        nc.scalar.copy(dst, pss[ch])

            def v_proj(jt):
                ps = pp.tile([P, 512], FP32, tag="ppv", name="ppv", bufs=2)
                for dt in range(ND):
                    nc.tensor.matmul(
                        ps,
                        xbf[dt][:, jt * P:(jt + 1) * P],
                        wv[dt],
                        start=(dt == 0),
                        stop=(dt == ND - 1),
                    )
                vsrc = ps.rearrange("p (h c) -> p h c", c=HD)
                vdst = vaug[jt].rearrange("p (h c) -> p h c", c=AUG)[:, :, 0:HD]
                if jt % 2 == 0:
                    nc.vector.tensor_copy(vdst, vsrc)
                else:
                    nc.scalar.copy(vdst, vsrc)

            def emit_dots(pool, tag, hp, jt, half, qh):
                """One head's dots [128 keys, 1024 queries] for key-tile jt,
                query-half qh. half 0 -> PE rows 0:64, half 1 -> rows 64:128."""
                d = pool.tile([P, NH], FP32, tag=tag, name="dots")
                jsl = slice(jt * P, (jt + 1) * P)
                rsl = slice(0, HD) if half == 0 else slice(HD, P)
                for ic in range(2):
                    isl = slice(ic * 512, (ic + 1) * 512)
                    qsl = slice(qh * NH + ic * 512, qh * NH + (ic + 1) * 512)
                    nc.tensor.matmul(
                        d[:, isl], kt[hp][rsl, jsl], qt[hp][rsl, qsl],
                        start=True, stop=True,
                    )
                return d

            def emit_exp(d, e_t):
                nc.scalar.activation(e_t, d, mybir.ActivationFunctionType.Exp,
                                     scale=SCALE)

            # ---- projection phase, with unit-0 (hp0, qh0) dots+exp
            # interleaved; exps buffered in SBUF for AV replay later.
            qk_proj(wq0, qt, 0, [0])
            qk_proj(wk0, kt, 0, [0])
            qk_proj(wk0, kt, 0, [1])

            e0 = [e0pool.tile([P, NH], BF16, tag=f"e0_{j}", name=f"e0_{j}")
                  for j in range(2 * NBUF)]
            # remaining weight-load jobs: q et1-3, k et1-3
            wjobs = [(0, e) for e in range(1, NE)] + \
                    [(512, e) for e in range(1, NE)]
            # proj chunk-jobs consumed by the filler scheduler below:
            # q et0 ch1 first (unit 1 = (hp0, qh1) needs it early).
            pjobs = [(0, 0, 1)]
            for (ebase, e) in wjobs:
                pjobs.append((ebase, e, 0))
                pjobs.append((ebase, e, 1))

            wtiles = {(0, 0): wq0, (512, 0): wk0}
            for key in wjobs:
                wtiles[key] = None
            wtiles[wjobs[0]] = load_we(*wjobs[0])

            # unit-0: dots+exp for jt 0..15 interleaved with v_proj and
            # q/k e-tile projections (PE fills while ACT chews exps).
            widx = 1
            for jt in range(16):
                if jt < NBUF:
                    dA = emit_dots(pp, 'pp', 0, jt, 0, 0)
                    dB = emit_dots(pp, 'pp', 0, jt, 1, 0)
                    emit_exp(dA, e0[2 * jt])
                    emit_exp(dB, e0[2 * jt + 1])
                v_proj(jt)
                if jt < len(pjobs):
                    ebase, e, ch = pjobs[jt]
                    w = wtiles[(ebase, e)]
                    if w is None:
                        w = load_we(ebase, e)
                        wtiles[(ebase, e)] = w
                    if widx < len(wjobs) and wtiles[wjobs[widx]] is None:
                        wtiles[wjobs[widx]] = load_we(*wjobs[widx])
                        widx += 1
                    tiles = qt if ebase == 0 else kt
                    qk_proj(w, tiles, e, [ch])

            pp_cm.__exit__(None, None, None)
            wpool_cm.__exit__(None, None, None)
            xpool_cm.__exit__(None, None, None)

            # ---- attention-phase pools
            late_cm = tc.tile_pool(name="late", bufs=1)
            late = late_cm.__enter__()
            aot = [late.tile([P, NQ], BF16, tag=f"ao{e}", name=f"ao{e}")
                   for e in range(NE)]
            late2_cm = tc.tile_pool(name="late2", bufs=1)
            late2 = late2_cm.__enter__()
            wo = [late2.tile([P, DIM], BF16, tag=f"wo{dt}", name=f"wo{dt}")
                  for dt in range(ND)]
            for dt in range(ND):
                nc.sync.dma_start(
                    out=wo[dt], in_=woutT.ap()[dt * P:(dt + 1) * P, :])

            def aoX(e):
                # select destinations reuse aot's storage: aot[hp] is dead
                # once both its halves are exchanged; dim-tile e = gg*4+hp
                # of MY tokens lands in aot[hp][:, gg-half]
                gg, hp = e // NE, e % NE
                return aot[hp][:, gg * NH:(gg + 1) * NH]

            norm_cm = tc.tile_pool(name="norm", bufs=1)
            norm = norm_cm.__enter__()
            pd_cm = tc.tile_pool(name="pd", bufs=2, space="PSUM")
            pd = pd_cm.__enter__()
            pav_cm = tc.tile_pool(name="pav", bufs=2, space="PSUM")
            pav = pav_cm.__enter__()

            def av_accum(av, e_t, jt, head):
                first, last = jt == 0, jt == 15
                for ic in range(2):
                    isl = slice(ic * 512, (ic + 1) * 512)
                    nc.tensor.matmul(
                        av[:, isl],
                        vaug[jt][:, head * AUG:(head + 1) * AUG],
                        e_t[:, isl],
                        start=first, stop=last,
                    )

            def heat(hp, n=10):
                # junk matmuls bridge the unit-boundary PE gap (HW DVFS).
                pb = pd.tile([P, NH], FP32, tag="pd", name="heat")
                for i in range(n):
                    nc.tensor.matmul(
                        pb[:, (i % 2) * 512:(i % 2) * 512 + 512],
                        kt[hp][:, 0:P], qt[hp][:, 0:512],
                        start=True, stop=True,
                    )

            def normalize(hp, qh, avA, avB, on_act=False):
                # per-unit softmax normalization (overlaps next unit).
                # on_act: route the bulk copies to the scalar engine (idle
                # after the final unit) to shorten the tail critical path.
                osl = slice(qh * NH, (qh + 1) * NH)
                cp = nc.scalar.copy if on_act else nc.vector.tensor_copy
                g = norm.tile([33, NH], FP32, tag="g", name="g")
                nc.vector.tensor_copy(g[0:1, :], avA[HD:AUG, :])
                nc.vector.tensor_copy(g[32:33, :], avB[HD:AUG, :])
                cp(aot[hp][0:HD, osl], avA[0:HD, :])
                cp(aot[hp][HD:P, osl], avB[0:HD, :])
                rp = norm.tile([33, NH], FP32, tag="rp", name="rp")
                nc.vector.reciprocal_approx_fast(rp, g)
                rbA = norm.tile([P, NH], FP32, tag="rb", name="rb", bufs=2)
                nc.gpsimd.partition_broadcast(rbA, rp[0:1, :])
                nc.vector.tensor_mul(
                    aot[hp][0:HD, osl], aot[hp][0:HD, osl], rbA[0:HD, :])
                tb = norm.tile([1, NH], FP32, tag="tb", name="tb")
                nc.vector.tensor_copy(tb, rp[32:33, :])
                rbB = norm.tile([P, NH], FP32, tag="rb", name="rb", bufs=2)
                nc.gpsimd.partition_broadcast(rbB, tb)
                nc.vector.tensor_mul(
                    aot[hp][HD:P, osl], aot[hp][HD:P, osl], rbB[HD:P, :])

            def exchange(hp, qh):
                # peer exchange of one (head-pair, query-half) of attention
                # outputs: rank-ordered AllGather through DRAM. Trigger only
                # — the CC runs ~8-13us async; the gathered-side DMAs and
                # selects are deferred (select_hp) until it is long done;
                # anything queued behind them on DVE/Sync would stall the
                # whole pipeline.
                osl = slice(qh * NH, (qh + 1) * NH)
                nc.sync.dma_start(out=ccin[hp][qh].ap(), in_=aot[hp][:, osl])
                nc.gpsimd.collective_compute(
                    "AllGather", mybir.AluOpType.bypass,
                    replica_groups=GROUPS,
                    ins=[ccin[hp][qh].ap()], outs=[ccout[hp][qh].ap()],
                )

            def select_hp(hp):
                # data-driven token-half select across the two half-gathers
                for gg in range(2):
                    rsl = slice(gg * P, (gg + 1) * P)
                    g0 = sb.tile([P, NH], BF16, tag="gx", name="gx", bufs=4)
                    g1 = sb.tile([P, NH], BF16, tag="gx", name="gx", bufs=4)
                    nc.sync.dma_start(out=g0, in_=ccout[hp][0].ap()[rsl, :])
                    nc.sync.dma_start(out=g1, in_=ccout[hp][1].ap()[rsl, :])
                    e = gg * NE + hp
                    t = sb.tile([P, NH], BF16, tag="selt", name="selt", bufs=2)
                    nc.vector.tensor_scalar(
                        t, g0, mbc[:, 0:1], None,
                        mybir.AluOpType.mult)
                    nc.vector.scalar_tensor_tensor(
                        aoX(e), g1, mbc[:, 1:2], t,
                        mybir.AluOpType.mult, mybir.AluOpType.add)

            # ---- attention phase: 8 units = (hp, qh)
            # unit-0 (hp0, qh0): pure AV replay of the 16 buffered jts.
            # u1's first dots go first so its exps start immediately and
            # the scalar engine never drains while the replay runs.
            avA = pav.tile([AUG, NH], FP32, tag="pav", name="av")
            avB = pav.tile([AUG, NH], FP32, tag="pav", name="av")
            dA_nx = emit_dots(pd, 'pd', 0, 0, 0, 1)
            dB_nx = emit_dots(pd, 'pd', 0, 0, 1, 1)
            for r in range(16):
                av_accum(avA, e0[2 * r], r, 0)
                av_accum(avB, e0[2 * r + 1], r, 1)
            normalize(0, 0, avA, avB)
            exchange(0, 0)

            # units 1-7: lookahead-dots pipeline.
            # CC(hp) fires after unit 2hp+1; its selects run inside unit
            # 2hp+4 (or the tail for hp2/hp3) when the CC is long done.
            sel_at = {4: 0, 6: 1, 7: 2}
            for u in range(1, 8):
                hp, qh = u // 2, u % 2
                avA = pav.tile([AUG, NH], FP32, tag="pav", name="av")
                avB = pav.tile([AUG, NH], FP32, tag="pav", name="av")
                dA, dB = dA_nx, dB_nx
                heat(hp)
                for jt in range(16):
                    eA = sb.tile([P, NH], BF16, tag="expT", name="expT", bufs=4)
                    eB = sb.tile([P, NH], BF16, tag="expT", name="expT", bufs=4)
                    emit_exp(dA, eA)
                    emit_exp(dB, eB)
                    if jt < 15:
                        dA = emit_dots(pd, 'pd', hp, jt + 1, 0, qh)
                    av_accum(avA, eA, jt, 2 * hp)
                    if jt < 15:
                        dB = emit_dots(pd, 'pd', hp, jt + 1, 1, qh)
                    av_accum(avB, eB, jt, 2 * hp + 1)
                    if jt == 15 and u < 7:
                        nhp, nqh = (u + 1) // 2, (u + 1) % 2
                        dA_nx = emit_dots(pd, 'pd', nhp, 0, 0, nqh)
                        dB_nx = emit_dots(pd, 'pd', nhp, 0, 1, nqh)
                    if jt == 8 and u in sel_at:
                        select_hp(sel_at[u])
                normalize(hp, qh, avA, avB, on_act=(u == 7))
                exchange(hp, qh)
            select_hp(3)

            pav_cm.__exit__(None, None, None)
            pd_cm.__exit__(None, None, None)
            norm_cm.__exit__(None, None, None)

            # ---- output projection + bias for my 1024 tokens.
            # aoX order: e = g*4 + hp ; wout rows for ao dim block e are
            # woutT rows [g*512 + hp*128 ...]. Accumulate hp3 tiles (3, 7)
            # last so the final exchange hides behind the others.
            et_order = [0, 1, 2, 4, 5, 6, 3, 7]
            po_cm = tc.tile_pool(name="po", bufs=8, space="PSUM")
            po = po_cm.__enter__()

            def wo_row(e):
                # ao dim-tile e = g*4+hp -> wout row-tile g*4+hp (same index)
                return wo[e]

            for fc in range(2):
                fsl = slice(fc * 512, (fc + 1) * 512)
                pss = {}
                for it in range(8):
                    pss[it] = po.tile([P, 512], FP32, tag="po", name="po")
                    for ei in range(6):
                        e = et_order[ei]
                        nc.tensor.matmul(
                            pss[it],
                            aoX(e)[:, it * P:(it + 1) * P],
                            wo_row(e)[:, fsl],
                            start=(ei == 0),
                            stop=False,
                        )
                for it in range(8):
                    for ei in range(6, 8):
                        e = et_order[ei]
                        nc.tensor.matmul(
                            pss[it],
                            aoX(e)[:, it * P:(it + 1) * P],
                            wo_row(e)[:, fsl],
                            start=False,
                            stop=(ei == 7),
                        )
                    osb = sb.tile([P, 512], FP32, tag="outsb", name="outsb",
                                  bufs=3)
                    nc.vector.tensor_add(osb, pss[it], bias_bc[:, fsl])
                    nc.sync.dma_start(
                        out=out.ap()[it * P:(it + 1) * P, fsl], in_=osb)
            po_cm.__exit__(None, None, None)
            late2_cm.__exit__(None, None, None)
            late_cm.__exit__(None, None, None)
            e0pool_cm.__exit__(None, None, None)

    nc.compile()
    return nc


def _get_nc():
    global _NC_CACHE
    if _NC_CACHE is None:
        _NC_CACHE = _build()
    return _NC_CACHE


def kernel(x, w_qkv, w_out, b_out):
    global LAST_RESULTS
    import ml_dtypes
    BF = ml_dtypes.bfloat16
    x = np.asarray(x, dtype=np.float32)
    w_qkv = np.asarray(w_qkv, dtype=np.float32)
    w_out = np.asarray(w_out, dtype=np.float32)
    b_out = np.asarray(b_out, dtype=np.float32)

    nc = _get_nc()

    wqkvT_full = w_qkv.T.astype(BF)  # [1024, 3072]
    woutT = np.ascontiguousarray(w_out.T.astype(BF))
    brow = np.ascontiguousarray(b_out.reshape(1, DIM))

    in_maps = []
    for c in range(N_CORES):
        b, g = divmod(c, 2)
        gsl = slice(g * 512, (g + 1) * 512)
        wslice = np.ascontiguousarray(np.concatenate([
            wqkvT_full[:, 0 * DIM:1 * DIM][:, gsl],
            wqkvT_full[:, 1 * DIM:2 * DIM][:, gsl],
            wqkvT_full[:, 2 * DIM:3 * DIM][:, gsl],
        ], axis=1))
        xTc = np.ascontiguousarray(x[b].T.astype(BF))
        m = np.zeros((1, 2), np.float32)
        m[0, g] = 1.0
        in_maps.append({
            "xT": xTc,
            "wqkvT": wslice,
            "woutT": woutT,
            "bout": brow,
            "mask": m,
        })

    res = run_bass_kernel_spmd(
        nc, in_maps, core_ids=list(range(N_CORES)), trace=TRACE
    )
    LAST_RESULTS = res

    out = np.empty((B, SEQ, DIM), dtype=np.float32)
    for c in range(N_CORES):
        b, g = divmod(c, 2)
        out[b, g * NH:(g + 1) * NH, :] = res.results[c]["out"]
    return out
